# revision 1
# baseline (speedup 1.0000x reference)
"""CTC loss (Keras ctc_batch_cost semantics) on 8 Trainium2 NeuronCores.

Strategy: data-parallel over the batch axis (64 sequences per core). The CTC
forward DP runs in the *linear* probability domain with periodic max-
renormalization (scaled forward algorithm), so each time step is 4 DVE ops on
a [64 batch-partitions, 161 extended-state] tile:

    y = m .* q[s-2]            (skip-transition mask multiply)
    x = q + q[s-1]
    u = x + y
    q' = (u [* 1/z]) .* g_t    (g_t = gathered per-state emission probs)

Emission probs g_t[b,s] = y_pred[b,t,ext[b,s]] are gathered per (b, t-chunk)
by GPSIMD indirect_copy in [t-partition, s-free] layout and transposed to the
chain's [b-partition, (t,s)-free] layout with SBUF->SBUF DMAs. Softmax
normalizers Z[b,t] = sum_c y_pred and the final log-combine are handled by
the scalar engine; the loss is

    loss[b] = sum_t ln Z[b,t] - sum_renorms ln z - ln(qT[S-1] + qT[S-2]).
"""

import functools
import os
import sys

import numpy as np

B, T, C, L = 512, 512, 128, 80
S = 2 * L + 1  # 161
BLANK = C - 1
EPS = 1e-7
NCORES = 8
BPC = B // NCORES  # 64 sequences per core
TC = 64  # time-chunk
NCHUNK = T // TC  # 8
NPAIR = BPC // 2  # 32 pair-tiles (2 sequences each) per chunk
IDXW = 12  # wrapped-index columns, padded even so slices stay 4B-aligned
RENORM = 8  # renormalize every 8 steps
SPAD = S + 2  # zero-padded state row
SG = S + 3  # gather width padded to a multiple of 4 (ISA requirement)


def _emit_kernel(ctx, tc, ypred, idxt, maskt, losst, variant="full"):
    import concourse.bass as bass  # noqa: F401
    import concourse.mybir as mybir

    nc = tc.nc
    f32 = mybir.dt.float32
    Alu = mybir.AluOpType
    Act = mybir.ActivationFunctionType

    singles = ctx.enter_context(tc.tile_pool(name="singles", bufs=1))
    ypool = ctx.enter_context(tc.tile_pool(name="ypool", bufs=2))
    gpool = ctx.enter_context(tc.tile_pool(name="gpool", bufs=2))
    g2pool = ctx.enter_context(tc.tile_pool(name="g2pool", bufs=4))
    zscr = ctx.enter_context(tc.tile_pool(name="zscr", bufs=2))
    small = ctx.enter_context(tc.tile_pool(name="small", bufs=2))
    finp = ctx.enter_context(tc.tile_pool(name="finp", bufs=8))
    psump = ctx.enter_context(tc.tile_pool(name="psum", bufs=1, space="PSUM"))

    # --- constants loaded once -------------------------------------------
    idx_sb = singles.tile([128, NPAIR * IDXW], mybir.dt.uint16)
    nc.sync.dma_start(out=idx_sb[:, :], in_=idxt)
    m_sb = singles.tile([BPC, S], f32)
    nc.sync.dma_start(out=m_sb[:, :], in_=maskt)
    # pre-touch idx on GPSIMD so no gather has to wait for its load DMA
    idx_scr = singles.tile([16, 1], mybir.dt.uint16)
    nc.gpsimd.tensor_copy(out=idx_scr[:, :], in_=idx_sb[0:16, 0:1])

    # Z accumulator: col = chunk*NPAIR + pair, value = sum_c y_pred for the
    # 64 t's x 2 b's living in that pair-tile's partitions.
    zbig = singles.tile([128, NCHUNK * NPAIR], f32)
    # half-selector for the final partition-axis reduction via PE
    halfsel = singles.tile([128, 2], f32)
    nc.vector.memset(halfsel[:, :], 0.0)
    nc.vector.memset(halfsel[0:64, 0:1], 1.0)
    nc.vector.memset(halfsel[64:128, 1:2], 1.0)

    # --- producers: load y chunks, Z row-sums, gathers, b<->t swap -------
    gtiles = []
    for ch in range(NCHUNK):
        t0 = ch * TC
        ytile = ypool.tile([128, NPAIR, C], f32, tag="ychunk")
        # one DMA per (chunk, pair-half): partition p=t, free=(pair,c)
        for h in range(2):
            nc.sync.dma_start(
                out=ytile[64 * h : 64 * h + 64, :, :],
                in_=ypred[h::2, t0 : t0 + TC, :].rearrange("j t c -> t j c"),
            )
        gtile = gpool.tile([BPC, TC * S], f32, tag="gchunk")
        for j in range(NPAIR):
            scr = zscr.tile([128, C], f32, tag="zscratch")
            nc.scalar.activation(
                out=scr[:, :],
                in_=ytile[:, j, :],
                func=Act.Copy,
                bias=EPS,
                accum_out=zbig[:, ch * NPAIR + j : ch * NPAIR + j + 1],
            )
            g2 = g2pool.tile([128, SG], f32, tag="g2")
            # Absorb the gather's sync waits (DMA RAW on ytile, swap-DMA WAR on
            # g2) into a cheap same-engine op: the IndirectCopy ISA struct has
            # too few sync-wait slots for Tile's generated waits.
            nc.gpsimd.tensor_copy(out=g2[0:16, 0:1], in_=ytile[0:16, j, 0:1])
            if variant == "nogather":
                nc.gpsimd.tensor_copy(out=g2[:, :], in_=ytile[:, j, 0:SG])
            else:
                nc.gpsimd.indirect_copy(
                    g2[:, :],
                    ytile[:, j, :],
                    idx_sb[:, j * IDXW : (j + 1) * IDXW],
                    True,
                )
            nc.sync.dma_start(out=gtile[2 * j : 2 * j + 2, :], in_=g2[:, 0:S])
        gtiles.append(gtile)

    # --- the DP chain -----------------------------------------------------
    qa = singles.tile([BPC, SPAD], f32)
    qb = singles.tile([BPC, SPAD], f32)
    xt = singles.tile([BPC, S], f32)
    yt = singles.tile([BPC, S], f32)
    ut = singles.tile([BPC, S], f32)
    nrenorm = (T - 2) // RENORM  # renorms measured at t%8==7, t<511
    zstash = singles.tile([BPC, nrenorm], f32)

    nc.vector.memset(qa[:, :], 0.0)
    nc.vector.memset(qb[:, 0:2], 0.0)
    # q0 = g_0 at s in {0,1}
    nc.vector.tensor_copy(out=qa[:, 2:4], in_=gtiles[0][:, 0:2])

    rz_tiles = {}
    cur, nxt = qa, qb
    nsteps = 1 if variant == "nochain" else T
    for t in range(1, nsteps):
        ch, toff = divmod(t, TC)
        g_slice = gtiles[ch][:, toff * S : (toff + 1) * S]
        nc.vector.tensor_tensor(out=yt[:, :], in0=m_sb[:, :], in1=cur[:, 0:S], op=Alu.mult)
        nc.vector.tensor_tensor(
            out=xt[:, :], in0=cur[:, 2:SPAD], in1=cur[:, 1 : S + 1], op=Alu.add
        )
        nc.vector.tensor_tensor(out=ut[:, :], in0=xt[:, :], in1=yt[:, :], op=Alu.add)
        k, phase = divmod(t, RENORM)
        if variant == "chain_tt":
            nc.vector.tensor_tensor(
                out=nxt[:, 2:SPAD], in0=ut[:, :], in1=g_slice, op=Alu.mult
            )
            if phase == RENORM - 1:
                nc.vector.tensor_scalar(
                    out=nxt[:, 2:SPAD],
                    in0=nxt[:, 2:SPAD],
                    scalar1=1e-10,
                    scalar2=1e10,
                    op0=Alu.max,
                    op1=Alu.min,
                )
        elif phase == RENORM - 1 and k < nrenorm:
            # note: tensor_tensor_reduce would fuse these two, but its ISA
            # encoding fails at runtime on this stack — keep them separate
            nc.vector.tensor_tensor(
                out=nxt[:, 2:SPAD], in0=ut[:, :], in1=g_slice, op=Alu.mult
            )
            nc.vector.reduce_max(
                out=zstash[:, k : k + 1],
                in_=nxt[:, 2:SPAD],
                axis=mybir.AxisListType.X,
            )
            rz = small.tile([BPC, 1], f32, tag="rz")
            nc.vector.reciprocal(out=rz[:, :], in_=zstash[:, k : k + 1])
            rz_tiles[k] = rz
        elif phase == 0 and (t // RENORM - 1) in rz_tiles:
            rz = rz_tiles[t // RENORM - 1]
            nc.vector.scalar_tensor_tensor(
                out=nxt[:, 2:SPAD],
                in0=ut[:, :],
                scalar=rz[:, :],
                in1=g_slice,
                op0=Alu.mult,
                op1=Alu.mult,
            )
        else:
            nc.vector.tensor_tensor(
                out=nxt[:, 2:SPAD], in0=ut[:, :], in1=g_slice, op=Alu.mult
            )
        cur, nxt = nxt, cur

    # --- epilogue: loss = W - r - ln(q[S-1] + q[S-2]) ---------------------
    if variant in ("nochain", "chain_tt"):
        # dummy values so the Ln/reduce epilogue stays finite
        nc.vector.memset(zstash[:, :], 1.0)
        if variant == "nochain":
            nc.vector.memset(cur[:, SPAD - 2 : SPAD], 1.0)
    qsum = finp.tile([BPC, 1], f32, tag="fin")
    nc.vector.tensor_tensor(
        out=qsum[:, :], in0=cur[:, SPAD - 1 : SPAD], in1=cur[:, SPAD - 2 : SPAD - 1], op=Alu.add
    )
    lnq = finp.tile([BPC, 1], f32, tag="fin")
    nc.scalar.activation(out=lnq[:, :], in_=qsum[:, :], func=Act.Ln)
    lnz = finp.tile([BPC, nrenorm], f32, tag="lnz")
    nc.scalar.activation(out=lnz[:, :], in_=zstash[:, :], func=Act.Ln)
    r = finp.tile([BPC, 1], f32, tag="fin")
    nc.vector.reduce_sum(out=r[:, :], in_=lnz[:, :], axis=mybir.AxisListType.X)

    lnZ = singles.tile([128, NCHUNK * NPAIR], f32)
    nc.scalar.activation(out=lnZ[:, :], in_=zbig[:, :], func=Act.Ln)
    wsum = singles.tile([128, NPAIR], f32)
    lnZ_v = lnZ[:, :].rearrange("p (c q) -> p q c", c=NCHUNK)
    nc.vector.reduce_sum(out=wsum[:, :], in_=lnZ_v, axis=mybir.AxisListType.X)
    psw = psump.tile([NPAIR, 2], f32)
    nc.tensor.matmul(psw[:, :], lhsT=wsum[:, :], rhs=halfsel[:, :], start=True, stop=True)
    wpsb = finp.tile([NPAIR, 2], f32, tag="wpsb")
    nc.vector.tensor_copy(out=wpsb[:, :], in_=psw[:, :])
    wb = finp.tile([BPC, 1], f32, tag="fin")
    nc.sync.dma_start(out=wb[:, :], in_=wpsb[:, :])

    t1 = finp.tile([BPC, 1], f32, tag="fin")
    nc.vector.tensor_tensor(out=t1[:, :], in0=wb[:, :], in1=r[:, :], op=Alu.subtract)
    lt = finp.tile([BPC, 1], f32, tag="fin")
    nc.vector.tensor_tensor(out=lt[:, :], in0=t1[:, :], in1=lnq[:, :], op=Alu.subtract)
    nc.sync.dma_start(out=losst, in_=lt[:, :])


@functools.lru_cache(maxsize=4)
def _build(variant="full"):
    from contextlib import ExitStack

    import concourse.bacc as bacc
    import concourse.mybir as mybir
    import concourse.tile as tile

    nc = bacc.Bacc(trn_type="TRN2", target_bir_lowering=False)
    ypred = nc.dram_tensor("y_pred", [BPC, T, C], mybir.dt.float32, kind="ExternalInput")
    idxt = nc.dram_tensor(
        "idx", [128, NPAIR * IDXW], mybir.dt.uint16, kind="ExternalInput"
    )
    maskt = nc.dram_tensor("mask", [BPC, S], mybir.dt.float32, kind="ExternalInput")
    losst = nc.dram_tensor("loss", [BPC, 1], mybir.dt.float32, kind="ExternalOutput")
    with tile.TileContext(nc) as tc:
        with ExitStack() as ctx:
            _emit_kernel(
                ctx, tc, ypred[:, :, :], idxt[:, :], maskt[:, :], losst[:, :], variant
            )
    nc.compile()
    return nc


def _host_prep(y_true):
    """Per-core wrapped gather indices and skip-transition masks."""
    y_true = np.asarray(y_true).astype(np.int64)
    ext = np.full((B, S), BLANK, dtype=np.int64)
    ext[:, 1::2] = y_true
    mask = np.zeros((B, S), dtype=np.float32)
    mask[:, 1] = 1.0
    lab = y_true
    neq = (lab[:, 1:] != lab[:, :-1]).astype(np.float32)
    mask[:, 3::2] = neq

    idx_all = []
    for k in range(NCORES):
        idx = np.zeros((128, NPAIR * IDXW), dtype=np.uint16)
        base = k * BPC
        p = np.arange(128)
        for j in range(NPAIR):
            b = base + 2 * j + (p >= 64).astype(np.int64)
            for f in range(IDXW):
                pos = f * 16 + (p % 16)
                valid = pos < S
                idx[p[valid], j * IDXW + f] = ext[b[valid], pos[valid]]
        idx_all.append(idx)
    return idx_all, mask


def kernel(y_true, y_pred):
    from concourse.bass_utils import run_bass_kernel_spmd

    y_pred = np.ascontiguousarray(np.asarray(y_pred), dtype=np.float32)
    idx_all, mask = _host_prep(y_true)

    nc = _build(os.environ.get("CTC_VARIANT", "full"))
    in_maps = []
    for k in range(NCORES):
        b0 = k * BPC
        in_maps.append(
            {
                "y_pred": np.ascontiguousarray(y_pred[b0 : b0 + BPC]),
                "idx": idx_all[k],
                "mask": np.ascontiguousarray(mask[b0 : b0 + BPC]),
            }
        )
    res = run_bass_kernel_spmd(
        nc,
        in_maps,
        core_ids=list(range(NCORES)),
        trace=bool(int(os.environ.get("CTC_TRACE", "0"))),
    )
    out = np.concatenate([r["loss"] for r in res.results], axis=0)
    if res.exec_time_ns is not None:
        print(f"HW exec time: {res.exec_time_ns} ns", file=sys.stderr)
    return out.astype(np.float32)



# revision 4
# speedup vs baseline: 1.0888x; 1.0888x over previous
"""CTC loss (Keras ctc_batch_cost semantics) on 8 Trainium2 NeuronCores.

Strategy: forward/backward time split on top of batch data-parallelism.
Cores 0-3 run the forward CTC DP on all 512 sequences (128 each, one per
SBUF partition) over t in [0, 256); cores 4-7 run the time+state-reversed
recursion over t in [511, 256] — which is algebraically the *same* program
applied to time-flipped y with state-flipped gather indices/masks, so one
SPMD NEFF serves both roles.  The halves meet in the middle:

    P[b] = sum_s u_fwd[b, s] * q_bwd[b, S-1-s]

where u_fwd is the forward pre-multiply band sum at the virtual step 256 and
q_bwd is the backward core's final chain state.  The host does this tiny
[B, S] dot plus the log/Z bookkeeping.

Per core the chain is 255 sequential steps of 4 DVE ops on a
[128 batch-partitions, <=161 extended-states] tile (band-limited: at step t
only states s < 2t+2 are reachable), in the linear probability domain with
max-renormalization every 16 steps:

    y = m .* q[s-2];  x = q + q[s-1];  u = x + y;  q' = (u [* 1/z]) .* g_t

Emission rows g_t[b, s] = y[b, t, ext[b, s]] are produced by GPSIMD
indirect_copy in a strip layout: gather instruction k covers sequences
8k..8k+7, with partition p = 16*c + tau holding TL=4 consecutive t's of
sequence 8k+c (the 16-partition GPSIMD cores share one wrapped index list =
one sequence).  This makes the y-load DMA 2KB-contiguous per partition and
the [t,s]->[b,(t,s)] transpose an SBUF->SBUF DMA of 2.6KB packets whose
destination partitions sweep all of SBUF (so all 16 SDMA engines share the
work — the v0 kernel's transposes serialized on 2 engines).

Softmax normalizers Z[b,t] = sum_c y are accumulated by the scalar engine
per (group, tl) row-block and reduced across the strip partitions with one
PE matmul against a 0/1 selector.
"""

import functools
import os
import sys

import numpy as np

B, T, C, L = 512, 512, 128, 80
S = 2 * L + 1  # 161
BLANK = C - 1
EPS = 1e-7
NCORES = 8
NPAIRS = NCORES // 2  # 4 fwd/bwd core pairs
BPC = 128  # sequences per core (one per partition)
THALF = T // 2  # 256 time steps per core
TC = 64  # time-chunk
NCHUNK = THALF // TC  # 4
NTAU = 16  # t-strips per gather instruction (one per partition of a group)
TL = TC // NTAU  # 4 consecutive t's per partition strip
NGRP = BPC // 8  # 16 gather groups of 8 sequences per chunk
SROW = S + 1  # 162: per-t gather row (161 states + 1 pad), keeps rows 4B-aligned
NIDX = TL * SROW  # 648 gather indices per instruction (multiple of 4)
IDXC = (NIDX + 15) // 16 + 1  # 42 wrapped-index columns (padded even)
RENORM = 16
NREN = (THALF - 2) // RENORM  # 15 renorms (t = 15, 31, .., 239)


def _emit_kernel(ctx, tc, y, idxt, maskt, cselt, qft, uft, wqt, rst):
    import concourse.bass as bass  # noqa: F401
    import concourse.mybir as mybir

    nc = tc.nc
    f32 = mybir.dt.float32
    Alu = mybir.AluOpType
    Act = mybir.ActivationFunctionType
    X = mybir.AxisListType.X

    singles = ctx.enter_context(tc.tile_pool(name="singles", bufs=1))
    ypool = ctx.enter_context(tc.tile_pool(name="ypool", bufs=4))
    gopool = ctx.enter_context(tc.tile_pool(name="gopool", bufs=4))
    gpool = ctx.enter_context(tc.tile_pool(name="gpool", bufs=2))
    zscr = ctx.enter_context(tc.tile_pool(name="zscr", bufs=2))
    small = ctx.enter_context(tc.tile_pool(name="small", bufs=2))
    finp = ctx.enter_context(tc.tile_pool(name="finp", bufs=8))
    psump = ctx.enter_context(tc.tile_pool(name="psum", bufs=1, space="PSUM"))

    # --- constants loaded once -------------------------------------------
    idx_sb = singles.tile([128, NGRP * IDXC], mybir.dt.uint16)
    nc.sync.dma_start(out=idx_sb[:, :], in_=idxt)
    m_sb = singles.tile([BPC, S], f32)
    nc.sync.dma_start(out=m_sb[:, :], in_=maskt)
    # pre-touch idx on GPSIMD so no gather waits on its load DMA
    idx_scr = singles.tile([16, 1], mybir.dt.uint16)
    nc.gpsimd.tensor_copy(out=idx_scr[:, :], in_=idx_sb[0:16, 0:1])

    # Z accumulator: col = ch*(NGRP*TL) + k*TL + tl; value = sum_c y+EPS over
    # the [128, C] row-block (partition p=(c,tau) holds seq 8k+c, t strip tau).
    zb = singles.tile([128, NCHUNK * NGRP * TL], f32)
    # 0/1 selector for the partition-axis reduction via PE: csel[p, m] = [p//16 == m]
    csel = singles.tile([128, 8], f32)
    nc.sync.dma_start(out=csel[:, :], in_=cselt)

    # --- producers: y strips -> Z sums + gathers -> b-major transpose ----
    gtiles = []
    for ch in range(NCHUNK):
        t0 = ch * TC
        gtile = gpool.tile([BPC, TC * SROW], f32, tag="gchunk")
        for k in range(NGRP):
            ytile = ypool.tile([128, TL * C], f32, tag="ychunk")
            # src [8 seqs, 16 tau, 2KB] -> [128, 512]: flat iteration orders
            # match (partition p = 16*c + tau), so the rank mismatch is fine.
            nc.sync.dma_start(
                out=ytile[:, :],
                in_=y[8 * k : 8 * k + 8, t0 : t0 + TC, :].rearrange(
                    "b (tau tl) c -> b tau (tl c)", tau=NTAU
                ),
            )
            for tl in range(TL):
                scr = zscr.tile([128, C], f32, tag="zscratch")
                nc.scalar.activation(
                    out=scr[:, :],
                    in_=ytile[:, tl * C : (tl + 1) * C],
                    func=Act.Copy,
                    bias=EPS,
                    accum_out=zb[
                        :, ch * NGRP * TL + k * TL + tl : ch * NGRP * TL + k * TL + tl + 1
                    ],
                )
            gout = gopool.tile([128, NIDX], f32, tag="gout")
            # Absorb Tile's sync waits (DMA RAW on ytile, WAR on gout) into a
            # cheap same-engine op: the IndirectCopy ISA struct has too few
            # sync-wait slots for the generated waits.
            nc.gpsimd.tensor_copy(out=gout[0:16, 0:1], in_=ytile[0:16, 0:1])
            nc.gpsimd.indirect_copy(
                gout[:, :],
                ytile[:, :],
                idx_sb[:, k * IDXC : (k + 1) * IDXC],
                True,
            )
            # strip transpose: src partition 16c+tau row (2592B) -> dest
            # partition 8k+c at col tau*NIDX; dest partitions sweep all 128
            # across k so every SDMA engine gets a share.
            nc.sync.dma_start(out=gtile[8 * k : 8 * k + 8, :], in_=gout[:, :])
        gtiles.append(gtile)

    # --- the DP chain -----------------------------------------------------
    SP = S + 2  # 163: cols 0,1 zero-pad, states at 2..162
    qa = singles.tile([BPC, SP], f32)
    qb = singles.tile([BPC, SP], f32)
    xt = singles.tile([BPC, S], f32)
    yt = singles.tile([BPC, S], f32)
    ut = singles.tile([BPC, S], f32)
    zst = singles.tile([BPC, NREN], f32)

    nc.vector.memset(qa[:, :], 0.0)
    nc.vector.memset(qb[:, :], 0.0)
    # q0 = g_0 at s in {0, 1}
    nc.vector.tensor_copy(out=qa[:, 2:4], in_=gtiles[0][:, 0:2])

    cur, nxt = qa, qb
    rz = None
    for t in range(1, THALF):
        ch, toff = divmod(t, TC)
        FD = min(S, 2 * t + 2)  # band: q_t[s] == 0 for s > 2t+1
        g_slice = gtiles[ch][:, toff * SROW : toff * SROW + FD]
        nc.vector.tensor_tensor(
            out=yt[:, 0:FD], in0=m_sb[:, 0:FD], in1=cur[:, 0:FD], op=Alu.mult
        )
        nc.vector.tensor_tensor(
            out=xt[:, 0:FD], in0=cur[:, 2 : 2 + FD], in1=cur[:, 1 : 1 + FD], op=Alu.add
        )
        nc.vector.tensor_tensor(
            out=ut[:, 0:FD], in0=xt[:, 0:FD], in1=yt[:, 0:FD], op=Alu.add
        )
        if rz is not None:
            nc.vector.scalar_tensor_tensor(
                out=nxt[:, 2 : 2 + FD],
                in0=ut[:, 0:FD],
                scalar=rz[:, :],
                in1=g_slice,
                op0=Alu.mult,
                op1=Alu.mult,
            )
            rz = None
        else:
            nc.vector.tensor_tensor(
                out=nxt[:, 2 : 2 + FD], in0=ut[:, 0:FD], in1=g_slice, op=Alu.mult
            )
        r = t // RENORM
        if t % RENORM == RENORM - 1 and r < NREN:
            nc.vector.reduce_max(out=zst[:, r : r + 1], in_=nxt[:, 2 : 2 + S], axis=X)
            rzt = small.tile([BPC, 1], f32, tag="rz")
            nc.vector.reciprocal(out=rzt[:, :], in_=zst[:, r : r + 1])
            rz = rzt
        cur, nxt = nxt, cur

    # u-extension: virtual step THALF without the g multiply
    nc.vector.tensor_tensor(out=yt[:, :], in0=m_sb[:, :], in1=cur[:, 0:S], op=Alu.mult)
    nc.vector.tensor_tensor(
        out=xt[:, :], in0=cur[:, 2 : 2 + S], in1=cur[:, 1 : 1 + S], op=Alu.add
    )
    nc.vector.tensor_tensor(out=ut[:, :], in0=xt[:, :], in1=yt[:, :], op=Alu.add)
    nc.sync.dma_start(out=uft, in_=ut[:, :])
    nc.sync.dma_start(out=qft, in_=cur[:, 2 : 2 + S])

    # --- W = sum_t ln Z, reduced to [8 c, 16 k] via PE --------------------
    lnzb = singles.tile([128, NCHUNK * NGRP * TL], f32)
    nc.scalar.activation(out=lnzb[:, :], in_=zb[:, :], func=Act.Ln)
    r1 = singles.tile([128, NGRP * NCHUNK], f32)  # [k, ch]
    lnv = lnzb[:, :].rearrange("p (ch k tl) -> p k ch tl", ch=NCHUNK, k=NGRP)
    r1v = r1[:, :].rearrange("p (k ch) -> p k ch", k=NGRP)
    nc.vector.reduce_sum(out=r1v, in_=lnv, axis=X)
    vsum = singles.tile([128, NGRP], f32)
    nc.vector.reduce_sum(
        out=vsum[:, :], in_=r1[:, :].rearrange("p (k ch) -> p k ch", k=NGRP), axis=X
    )
    psw = psump.tile([8, NGRP], f32)
    nc.tensor.matmul(psw[:, :], lhsT=csel[:, :], rhs=vsum[:, :], start=True, stop=True)
    wqs = finp.tile([8, NGRP], f32, tag="fin")
    nc.vector.tensor_copy(out=wqs[:, :], in_=psw[:, :])
    nc.sync.dma_start(out=wqt, in_=wqs[:, :])

    # --- R = sum ln z over renorms ---------------------------------------
    lnz2 = finp.tile([BPC, NREN], f32, tag="lnz")
    nc.scalar.activation(out=lnz2[:, :], in_=zst[:, :], func=Act.Ln)
    rs = finp.tile([BPC, 1], f32, tag="fin")
    nc.vector.reduce_sum(out=rs[:, :], in_=lnz2[:, :], axis=X)
    nc.sync.dma_start(out=rst, in_=rs[:, :])


@functools.lru_cache(maxsize=2)
def _build():
    from contextlib import ExitStack

    import concourse.bacc as bacc
    import concourse.mybir as mybir
    import concourse.tile as tile

    nc = bacc.Bacc(trn_type="TRN2", target_bir_lowering=False)
    y = nc.dram_tensor("y", [BPC, THALF, C], mybir.dt.float32, kind="ExternalInput")
    idxt = nc.dram_tensor(
        "idx", [128, NGRP * IDXC], mybir.dt.uint16, kind="ExternalInput"
    )
    maskt = nc.dram_tensor("mask", [BPC, S], mybir.dt.float32, kind="ExternalInput")
    cselt = nc.dram_tensor("csel", [128, 8], mybir.dt.float32, kind="ExternalInput")
    qft = nc.dram_tensor("qf", [BPC, S], mybir.dt.float32, kind="ExternalOutput")
    uft = nc.dram_tensor("uf", [BPC, S], mybir.dt.float32, kind="ExternalOutput")
    wqt = nc.dram_tensor("wq", [8, NGRP], mybir.dt.float32, kind="ExternalOutput")
    rst = nc.dram_tensor("rsum", [BPC, 1], mybir.dt.float32, kind="ExternalOutput")
    with tile.TileContext(nc) as tc:
        with ExitStack() as ctx:
            _emit_kernel(
                ctx, tc, y[:, :, :], idxt[:, :], maskt[:, :], cselt[:, :],
                qft[:, :], uft[:, :], wqt[:, :], rst[:, :],
            )
    nc.compile()
    return nc


def _wrap_idx(ext_core):
    """ext_core: [BPC, S] gather classes for one core. Returns the wrapped
    uint16 index table [128, NGRP*IDXC] for the strip gathers."""
    idx = np.zeros((128, NGRP * IDXC), dtype=np.uint16)
    js = np.arange(NIDX)
    rows = js % 16
    cols = js // 16
    lst = np.zeros(NIDX, dtype=np.uint16)
    for k in range(NGRP):
        for c in range(8):
            b = 8 * k + c
            for tl in range(TL):
                lst[tl * SROW : tl * SROW + S] = tl * C + ext_core[b]
                lst[tl * SROW + S] = 0
            idx[16 * c + rows, k * IDXC + cols] = lst
    return idx


def _host_prep(y_true):
    """Per-core gather index tables and skip-transition masks (fwd + bwd)."""
    y_true = np.asarray(y_true).astype(np.int64)
    ext = np.full((B, S), BLANK, dtype=np.int64)
    ext[:, 1::2] = y_true
    mask = np.zeros((B, S), dtype=np.float32)
    mask[:, 1] = 1.0
    mask[:, 3::2] = (y_true[:, 1:] != y_true[:, :-1]).astype(np.float32)
    # backward (time+state reversed) tables
    ext_b = ext[:, ::-1]
    mask_b = np.ones((B, S), dtype=np.float32)
    sig = np.arange(2, S)
    mask_b[:, 2:] = mask[:, S + 1 - sig]
    idxs, masks = [], []
    for i in range(NPAIRS):
        b0 = i * BPC
        idxs.append(_wrap_idx(ext[b0 : b0 + BPC]))
        masks.append(np.ascontiguousarray(mask[b0 : b0 + BPC]))
    for i in range(NPAIRS):
        b0 = i * BPC
        idxs.append(_wrap_idx(ext_b[b0 : b0 + BPC]))
        masks.append(np.ascontiguousarray(mask_b[b0 : b0 + BPC]))
    return idxs, masks


def kernel(y_true, y_pred):
    from concourse.bass_utils import run_bass_kernel_spmd

    y_pred = np.asarray(y_pred, dtype=np.float32)
    idxs, masks = _host_prep(y_true)
    csel = np.zeros((128, 8), dtype=np.float32)
    csel[np.arange(128), np.arange(128) // 16] = 1.0

    nc = _build()
    in_maps = []
    for i in range(NPAIRS):  # forward cores: t in [0, 256)
        b0 = i * BPC
        in_maps.append(
            {
                "y": np.ascontiguousarray(y_pred[b0 : b0 + BPC, :THALF]),
                "idx": idxs[i],
                "mask": masks[i],
                "csel": csel,
            }
        )
    for i in range(NPAIRS):  # backward cores: t flipped from [256, 512)
        b0 = i * BPC
        in_maps.append(
            {
                "y": np.ascontiguousarray(y_pred[b0 : b0 + BPC, THALF:][:, ::-1]),
                "idx": idxs[NPAIRS + i],
                "mask": masks[NPAIRS + i],
                "csel": csel,
            }
        )
    res = run_bass_kernel_spmd(
        nc,
        in_maps,
        core_ids=list(range(NCORES)),
        trace=bool(int(os.environ.get("CTC_TRACE", "0"))),
    )
    bb = np.arange(BPC)
    out = np.empty((B, 1), dtype=np.float32)
    for i in range(NPAIRS):
        rf = res.results[i]
        rb = res.results[NPAIRS + i]
        w_f = rf["wq"][bb % 8, bb // 8].astype(np.float64)
        w_b = rb["wq"][bb % 8, bb // 8].astype(np.float64)
        dot = np.sum(
            rf["uf"].astype(np.float64) * rb["qf"][:, ::-1].astype(np.float64), axis=1
        )
        loss = (
            (w_f + w_b)
            - (rf["rsum"][:, 0].astype(np.float64) + rb["rsum"][:, 0].astype(np.float64))
            - np.log(dot)
        )
        out[i * BPC : (i + 1) * BPC, 0] = loss.astype(np.float32)
    if res.exec_time_ns is not None:
        print(f"HW exec time: {res.exec_time_ns} ns", file=sys.stderr)
    return out


# revision 6
# speedup vs baseline: 1.1839x; 1.0873x over previous
"""CTC loss (Keras ctc_batch_cost semantics) on 8 Trainium2 NeuronCores.

Strategy: forward/backward time split on top of batch data-parallelism.
Cores 0-3 run the forward CTC DP on all 512 sequences (128 each, one per
SBUF partition) over t in [0, 256); cores 4-7 run the time+state-reversed
recursion over t in [511, 256] — which is algebraically the *same* program
applied to time-flipped y with state-flipped gather indices/masks, so one
SPMD NEFF serves both roles.  The halves meet in the middle:

    P[b] = sum_s u_fwd[b, s] * q_bwd[b, S-1-s]

where u_fwd is the forward pre-multiply band sum at the virtual step 256 and
q_bwd is the backward core's final chain state.  The host does this tiny
[B, S] dot plus the log/Z bookkeeping.

Per core the chain is 255 sequential steps of 4 DVE ops on a
[128 batch-partitions, <=161 extended-states] tile (band-limited: at step t
only states s < 2t+2 are reachable), in the linear probability domain with
max-renormalization every 16 steps:

    y = m .* q[s-2];  x = q + q[s-1];  u = x + y;  q' = (u [* 1/z]) .* g_t

Emission rows g_t[b, s] = y[b, t, ext[b, s]] are produced by GPSIMD
indirect_copy in a strip layout: gather instruction k covers sequences
8k..8k+7, with partition p = 16*c + tau holding TL=4 consecutive t's of
sequence 8k+c (the 16-partition GPSIMD cores share one wrapped index list =
one sequence).  This makes the y-load DMA 2KB-contiguous per partition and
the [t,s]->[b,(t,s)] transpose an SBUF->SBUF DMA of 2.6KB packets whose
destination partitions sweep all of SBUF (so all 16 SDMA engines share the
work — the v0 kernel's transposes serialized on 2 engines).

Softmax normalizers Z[b,t] = sum_c y are accumulated by the scalar engine
per (group, tl) row-block and reduced across the strip partitions with one
PE matmul against a 0/1 selector.
"""

import functools
import os
import sys

import numpy as np

B, T, C, L = 512, 512, 128, 80
S = 2 * L + 1  # 161
BLANK = C - 1
EPS = 1e-7
NCORES = 8
NPAIRS = NCORES // 2  # 4 fwd/bwd core pairs
BPC = 128  # sequences per core (one per partition)
THALF = T // 2  # 256 time steps per core
TC = 64  # time-chunk
NCHUNK = THALF // TC  # 4
NTAU = 16  # t-strips per gather instruction (one per partition of a group)
TL = TC // NTAU  # 4 consecutive t's per partition strip
NGRP = BPC // 8  # 16 gather groups of 8 sequences per chunk
SROW = S + 1  # 162: per-t gather row (161 states + 1 pad), keeps rows 4B-aligned
NIDX = TL * SROW  # 648 gather indices per instruction (multiple of 4)
IDXC = (NIDX + 15) // 16 + 1  # 42 wrapped-index columns (padded even)
RENORM = 16
NREN = (THALF - 2) // RENORM  # 15 renorms (t = 15, 31, .., 239)


def _emit_kernel(ctx, tc, y, idxt, maskt, cselt, qft, uft, wqt, rst):
    import concourse.bass as bass  # noqa: F401
    import concourse.mybir as mybir

    nc = tc.nc
    f32 = mybir.dt.float32
    Alu = mybir.AluOpType
    Act = mybir.ActivationFunctionType
    X = mybir.AxisListType.X

    singles = ctx.enter_context(tc.tile_pool(name="singles", bufs=1))
    ypool = ctx.enter_context(tc.tile_pool(name="ypool", bufs=NGRP + 2))
    gopool = ctx.enter_context(tc.tile_pool(name="gopool", bufs=4))
    gpool = ctx.enter_context(tc.tile_pool(name="gpool", bufs=2))
    zscr = ctx.enter_context(tc.tile_pool(name="zscr", bufs=2))
    small = ctx.enter_context(tc.tile_pool(name="small", bufs=2))
    finp = ctx.enter_context(tc.tile_pool(name="finp", bufs=8))
    psump = ctx.enter_context(tc.tile_pool(name="psum", bufs=1, space="PSUM"))

    # --- constants loaded once -------------------------------------------
    idx_sb = singles.tile([128, NGRP * IDXC], mybir.dt.uint16)
    nc.sync.dma_start(out=idx_sb[:, :], in_=idxt)
    m_sb = singles.tile([BPC, S], f32)
    nc.sync.dma_start(out=m_sb[:, :], in_=maskt)
    # pre-touch idx on GPSIMD so no gather waits on its load DMA
    idx_scr = singles.tile([16, 1], mybir.dt.uint16)
    nc.gpsimd.tensor_copy(out=idx_scr[:, :], in_=idx_sb[0:16, 0:1])

    # Z accumulator: col = ch*(NGRP*TL) + k*TL + tl; value = sum_c y+EPS over
    # the [128, C] row-block (partition p=(c,tau) holds seq 8k+c, t strip tau).
    zb = singles.tile([128, NCHUNK * NGRP * TL], f32)
    # 0/1 selector for the partition-axis reduction via PE: csel[p, m] = [p//16 == m]
    csel = singles.tile([128, 8], f32)
    nc.sync.dma_start(out=csel[:, :], in_=cselt)

    # --- producers: y strips -> Z sums + gathers -> b-major transpose ----
    gtiles = []
    for ch in range(NCHUNK):
        t0 = ch * TC
        gtile = gpool.tile([BPC, TC * SROW], f32, tag="gchunk")
        # Issue the whole chunk's y loads before any dependent transpose: the
        # sync engine executes DMAs FIFO, so a transpose's wait-on-gather must
        # not sit between y loads or it stalls the producer pipeline.
        ytiles = []
        for k in range(NGRP):
            ytile = ypool.tile([128, TL * C], f32, tag="ychunk")
            # src [8 seqs, 16 tau, 2KB] -> [128, 512]: flat iteration orders
            # match (partition p = 16*c + tau), so the rank mismatch is fine.
            nc.sync.dma_start(
                out=ytile[:, :],
                in_=y[8 * k : 8 * k + 8, t0 : t0 + TC, :].rearrange(
                    "b (tau tl) c -> b tau (tl c)", tau=NTAU
                ),
            )
            ytiles.append(ytile)
        for k in range(NGRP):
            ytile = ytiles[k]
            for tl in range(TL):
                scr = zscr.tile([128, C], f32, tag="zscratch")
                nc.scalar.activation(
                    out=scr[:, :],
                    in_=ytile[:, tl * C : (tl + 1) * C],
                    func=Act.Copy,
                    bias=EPS,
                    accum_out=zb[
                        :, ch * NGRP * TL + k * TL + tl : ch * NGRP * TL + k * TL + tl + 1
                    ],
                )
            gout = gopool.tile([128, NIDX], f32, tag="gout")
            # Absorb Tile's sync waits (DMA RAW on ytile, WAR on gout) into a
            # cheap same-engine op: the IndirectCopy ISA struct has too few
            # sync-wait slots for the generated waits.
            nc.gpsimd.tensor_copy(out=gout[0:16, 0:1], in_=ytile[0:16, 0:1])
            nc.gpsimd.indirect_copy(
                gout[:, :],
                ytile[:, :],
                idx_sb[:, k * IDXC : (k + 1) * IDXC],
                True,
            )
            # strip transpose: src partition 16c+tau row (2592B) -> dest
            # partition 8k+c at col tau*NIDX; dest partitions sweep all 128
            # across k so every SDMA engine gets a share.
            nc.sync.dma_start(out=gtile[8 * k : 8 * k + 8, :], in_=gout[:, :])
        gtiles.append(gtile)

    # --- the DP chain -----------------------------------------------------
    SP = S + 2  # 163: cols 0,1 zero-pad, states at 2..162
    qa = singles.tile([BPC, SP], f32)
    qb = singles.tile([BPC, SP], f32)
    xt = singles.tile([BPC, S], f32)
    yt = singles.tile([BPC, S], f32)
    ut = singles.tile([BPC, S], f32)
    zst = singles.tile([BPC, NREN], f32)

    nc.vector.memset(qa[:, :], 0.0)
    nc.vector.memset(qb[:, :], 0.0)
    # q0 = g_0 at s in {0, 1}
    nc.vector.tensor_copy(out=qa[:, 2:4], in_=gtiles[0][:, 0:2])

    cur, nxt = qa, qb
    rz = None
    for t in range(1, THALF):
        ch, toff = divmod(t, TC)
        FD = min(S, 2 * t + 2)  # band: q_t[s] == 0 for s > 2t+1
        g_slice = gtiles[ch][:, toff * SROW : toff * SROW + FD]
        nc.vector.tensor_tensor(
            out=yt[:, 0:FD], in0=m_sb[:, 0:FD], in1=cur[:, 0:FD], op=Alu.mult
        )
        nc.vector.tensor_tensor(
            out=xt[:, 0:FD], in0=cur[:, 2 : 2 + FD], in1=cur[:, 1 : 1 + FD], op=Alu.add
        )
        nc.vector.tensor_tensor(
            out=ut[:, 0:FD], in0=xt[:, 0:FD], in1=yt[:, 0:FD], op=Alu.add
        )
        if rz is not None:
            nc.vector.scalar_tensor_tensor(
                out=nxt[:, 2 : 2 + FD],
                in0=ut[:, 0:FD],
                scalar=rz[:, :],
                in1=g_slice,
                op0=Alu.mult,
                op1=Alu.mult,
            )
            rz = None
        else:
            nc.vector.tensor_tensor(
                out=nxt[:, 2 : 2 + FD], in0=ut[:, 0:FD], in1=g_slice, op=Alu.mult
            )
        r = t // RENORM
        if t % RENORM == RENORM - 1 and r < NREN:
            nc.vector.reduce_max(out=zst[:, r : r + 1], in_=nxt[:, 2 : 2 + S], axis=X)
            rzt = small.tile([BPC, 1], f32, tag="rz")
            nc.vector.reciprocal(out=rzt[:, :], in_=zst[:, r : r + 1])
            rz = rzt
        cur, nxt = nxt, cur

    # u-extension: virtual step THALF without the g multiply
    nc.vector.tensor_tensor(out=yt[:, :], in0=m_sb[:, :], in1=cur[:, 0:S], op=Alu.mult)
    nc.vector.tensor_tensor(
        out=xt[:, :], in0=cur[:, 2 : 2 + S], in1=cur[:, 1 : 1 + S], op=Alu.add
    )
    nc.vector.tensor_tensor(out=ut[:, :], in0=xt[:, :], in1=yt[:, :], op=Alu.add)
    nc.sync.dma_start(out=uft, in_=ut[:, :])
    nc.sync.dma_start(out=qft, in_=cur[:, 2 : 2 + S])

    # --- W = sum_t ln Z, reduced to [8 c, 16 k] via PE --------------------
    lnzb = singles.tile([128, NCHUNK * NGRP * TL], f32)
    nc.scalar.activation(out=lnzb[:, :], in_=zb[:, :], func=Act.Ln)
    r1 = singles.tile([128, NGRP * NCHUNK], f32)  # [k, ch]
    lnv = lnzb[:, :].rearrange("p (ch k tl) -> p k ch tl", ch=NCHUNK, k=NGRP)
    r1v = r1[:, :].rearrange("p (k ch) -> p k ch", k=NGRP)
    nc.vector.reduce_sum(out=r1v, in_=lnv, axis=X)
    vsum = singles.tile([128, NGRP], f32)
    nc.vector.reduce_sum(
        out=vsum[:, :], in_=r1[:, :].rearrange("p (k ch) -> p k ch", k=NGRP), axis=X
    )
    psw = psump.tile([8, NGRP], f32)
    nc.tensor.matmul(psw[:, :], lhsT=csel[:, :], rhs=vsum[:, :], start=True, stop=True)
    wqs = finp.tile([8, NGRP], f32, tag="fin")
    nc.vector.tensor_copy(out=wqs[:, :], in_=psw[:, :])
    nc.sync.dma_start(out=wqt, in_=wqs[:, :])

    # --- R = sum ln z over renorms ---------------------------------------
    lnz2 = finp.tile([BPC, NREN], f32, tag="lnz")
    nc.scalar.activation(out=lnz2[:, :], in_=zst[:, :], func=Act.Ln)
    rs = finp.tile([BPC, 1], f32, tag="fin")
    nc.vector.reduce_sum(out=rs[:, :], in_=lnz2[:, :], axis=X)
    nc.sync.dma_start(out=rst, in_=rs[:, :])


@functools.lru_cache(maxsize=2)
def _build():
    from contextlib import ExitStack

    import concourse.bacc as bacc
    import concourse.mybir as mybir
    import concourse.tile as tile

    nc = bacc.Bacc(trn_type="TRN2", target_bir_lowering=False)
    y = nc.dram_tensor("y", [BPC, THALF, C], mybir.dt.float32, kind="ExternalInput")
    idxt = nc.dram_tensor(
        "idx", [128, NGRP * IDXC], mybir.dt.uint16, kind="ExternalInput"
    )
    maskt = nc.dram_tensor("mask", [BPC, S], mybir.dt.float32, kind="ExternalInput")
    cselt = nc.dram_tensor("csel", [128, 8], mybir.dt.float32, kind="ExternalInput")
    qft = nc.dram_tensor("qf", [BPC, S], mybir.dt.float32, kind="ExternalOutput")
    uft = nc.dram_tensor("uf", [BPC, S], mybir.dt.float32, kind="ExternalOutput")
    wqt = nc.dram_tensor("wq", [8, NGRP], mybir.dt.float32, kind="ExternalOutput")
    rst = nc.dram_tensor("rsum", [BPC, 1], mybir.dt.float32, kind="ExternalOutput")
    with tile.TileContext(nc) as tc:
        with ExitStack() as ctx:
            _emit_kernel(
                ctx, tc, y[:, :, :], idxt[:, :], maskt[:, :], cselt[:, :],
                qft[:, :], uft[:, :], wqt[:, :], rst[:, :],
            )
    nc.compile()
    return nc


def _wrap_idx(ext_core):
    """ext_core: [BPC, S] gather classes for one core. Returns the wrapped
    uint16 index table [128, NGRP*IDXC] for the strip gathers."""
    idx = np.zeros((128, NGRP * IDXC), dtype=np.uint16)
    js = np.arange(NIDX)
    rows = js % 16
    cols = js // 16
    lst = np.zeros(NIDX, dtype=np.uint16)
    for k in range(NGRP):
        for c in range(8):
            b = 8 * k + c
            for tl in range(TL):
                lst[tl * SROW : tl * SROW + S] = tl * C + ext_core[b]
                lst[tl * SROW + S] = 0
            idx[16 * c + rows, k * IDXC + cols] = lst
    return idx


def _host_prep(y_true):
    """Per-core gather index tables and skip-transition masks (fwd + bwd)."""
    y_true = np.asarray(y_true).astype(np.int64)
    ext = np.full((B, S), BLANK, dtype=np.int64)
    ext[:, 1::2] = y_true
    mask = np.zeros((B, S), dtype=np.float32)
    mask[:, 1] = 1.0
    mask[:, 3::2] = (y_true[:, 1:] != y_true[:, :-1]).astype(np.float32)
    # backward (time+state reversed) tables
    ext_b = ext[:, ::-1]
    mask_b = np.ones((B, S), dtype=np.float32)
    sig = np.arange(2, S)
    mask_b[:, 2:] = mask[:, S + 1 - sig]
    idxs, masks = [], []
    for i in range(NPAIRS):
        b0 = i * BPC
        idxs.append(_wrap_idx(ext[b0 : b0 + BPC]))
        masks.append(np.ascontiguousarray(mask[b0 : b0 + BPC]))
    for i in range(NPAIRS):
        b0 = i * BPC
        idxs.append(_wrap_idx(ext_b[b0 : b0 + BPC]))
        masks.append(np.ascontiguousarray(mask_b[b0 : b0 + BPC]))
    return idxs, masks


def kernel(y_true, y_pred):
    from concourse.bass_utils import run_bass_kernel_spmd

    y_pred = np.asarray(y_pred, dtype=np.float32)
    idxs, masks = _host_prep(y_true)
    csel = np.zeros((128, 8), dtype=np.float32)
    csel[np.arange(128), np.arange(128) // 16] = 1.0

    nc = _build()
    in_maps = []
    for i in range(NPAIRS):  # forward cores: t in [0, 256)
        b0 = i * BPC
        in_maps.append(
            {
                "y": np.ascontiguousarray(y_pred[b0 : b0 + BPC, :THALF]),
                "idx": idxs[i],
                "mask": masks[i],
                "csel": csel,
            }
        )
    for i in range(NPAIRS):  # backward cores: t flipped from [256, 512)
        b0 = i * BPC
        in_maps.append(
            {
                "y": np.ascontiguousarray(y_pred[b0 : b0 + BPC, THALF:][:, ::-1]),
                "idx": idxs[NPAIRS + i],
                "mask": masks[NPAIRS + i],
                "csel": csel,
            }
        )
    res = run_bass_kernel_spmd(
        nc,
        in_maps,
        core_ids=list(range(NCORES)),
        trace=bool(int(os.environ.get("CTC_TRACE", "0"))),
    )
    bb = np.arange(BPC)
    out = np.empty((B, 1), dtype=np.float32)
    for i in range(NPAIRS):
        rf = res.results[i]
        rb = res.results[NPAIRS + i]
        w_f = rf["wq"][bb % 8, bb // 8].astype(np.float64)
        w_b = rb["wq"][bb % 8, bb // 8].astype(np.float64)
        dot = np.sum(
            rf["uf"].astype(np.float64) * rb["qf"][:, ::-1].astype(np.float64), axis=1
        )
        loss = (
            (w_f + w_b)
            - (rf["rsum"][:, 0].astype(np.float64) + rb["rsum"][:, 0].astype(np.float64))
            - np.log(dot)
        )
        out[i * BPC : (i + 1) * BPC, 0] = loss.astype(np.float32)
    if res.exec_time_ns is not None:
        print(f"HW exec time: {res.exec_time_ns} ns", file=sys.stderr)
    return out


# revision 12
# speedup vs baseline: 3.4337x; 2.9004x over previous
"""CTC loss (Keras ctc_batch_cost semantics) on 8 Trainium2 NeuronCores.

Strategy: forward/backward time split on top of batch data-parallelism.
Cores 0-3 run the forward CTC DP on all 512 sequences (128 each, one per
SBUF partition) over t in [0, 256); cores 4-7 run the time+state-reversed
recursion over t in [511, 256] — which is algebraically the *same* program
applied to time-flipped y with state-flipped gather indices/masks, so one
SPMD NEFF serves both roles.  The halves meet in the middle:

    P[b] = sum_s u_fwd[b, s] * q_bwd[b, S-1-s]

where u_fwd is the forward pre-multiply band sum at the virtual step 256 and
q_bwd is the backward core's final chain state.  The host does this tiny
[B, S] dot plus the log/Z bookkeeping.

Per core the chain is 255 sequential steps of 4 DVE ops on a
[128 batch-partitions, <=161 extended-states] tile (band-limited: at step t
only states s < 2t+2 are reachable), in the linear probability domain with
max-renormalization every 16 steps:

    y = m .* q[s-2];  x = q + q[s-1];  u = x + y;  q' = (u [* 1/z]) .* g_t

Emission rows g_t[b, s] = y[b, t, ext[b, s]] are produced by GPSIMD
indirect_copy in a strip layout: gather instruction k covers sequences
8k..8k+7, with partition p = 16*c + tau holding TL=4 consecutive t's of
sequence 8k+c (the 16-partition GPSIMD cores share one wrapped index list =
one sequence).  This makes the y-load DMA 2KB-contiguous per partition and
the [t,s]->[b,(t,s)] transpose an SBUF->SBUF DMA of 2.6KB packets whose
destination partitions sweep all of SBUF (so all 16 SDMA engines share the
work — the v0 kernel's transposes serialized on 2 engines).

Softmax normalizers Z[b,t] = sum_c y are accumulated by the scalar engine
per (group, tl) row-block and reduced across the strip partitions with one
PE matmul against a 0/1 selector.
"""

import functools
import os
import sys

import numpy as np

B, T, C, L = 512, 512, 128, 80
S = 2 * L + 1  # 161
BLANK = C - 1
EPS = 1e-7
NCORES = 8
NPAIRS = NCORES // 2  # 4 fwd/bwd core pairs
BPC = 128  # sequences per core (one per partition)
THALF = T // 2  # 256 time steps per core
TC = 128  # time-chunk
NCHUNK = THALF // TC  # 2
NTAU = 16  # t-strips per gather instruction (one per partition of a group)
TL = TC // NTAU  # 8 consecutive t's per partition strip
NGRP = BPC // 8  # 16 gather groups of 8 sequences per chunk
SROW = S + 1  # 162: per-t gather row (161 states + 1 pad), keeps rows 4B-aligned
NIDX = TL * SROW  # 1296 gathered values per (group, chunk)
NGH = 2  # gather halves: the IC ISA dst pattern tops out below 1296 elems
NIDXH = NIDX // NGH  # 648 indices per indirect_copy (multiple of 4)
IDXC = (NIDXH + 15) // 16 + 1  # 42 wrapped-index columns (padded even)
RENORM = 32  # bf16's exponent range comfortably covers 32-step products
NREN = (THALF - 2) // RENORM  # 7 renorms (t = 31, 63, .., 223)


def _emit_kernel(ctx, tc, y, idxt, maskt, cselt, qft, uft, wqt, rst):
    import concourse.bass as bass  # noqa: F401
    import concourse.mybir as mybir

    nc = tc.nc
    f32 = mybir.dt.float32
    bf16 = mybir.dt.bfloat16
    Alu = mybir.AluOpType
    Act = mybir.ActivationFunctionType
    X = mybir.AxisListType.X

    singles = ctx.enter_context(tc.tile_pool(name="singles", bufs=1))
    ypool = ctx.enter_context(tc.tile_pool(name="ypool", bufs=6))
    gopool = ctx.enter_context(tc.tile_pool(name="gopool", bufs=4))
    gpool = ctx.enter_context(tc.tile_pool(name="gpool", bufs=2))
    zscr = ctx.enter_context(tc.tile_pool(name="zscr", bufs=2))
    small = ctx.enter_context(tc.tile_pool(name="small", bufs=2))
    finp = ctx.enter_context(tc.tile_pool(name="finp", bufs=8))
    psump = ctx.enter_context(tc.tile_pool(name="psum", bufs=1, space="PSUM"))

    # --- constants loaded once -------------------------------------------
    idx_sb = singles.tile([128, NGRP * NGH * IDXC], mybir.dt.uint16)
    nc.sync.dma_start(out=idx_sb[:, :], in_=idxt)
    m_sb = singles.tile([BPC, S], f32)
    nc.sync.dma_start(out=m_sb[:, :], in_=maskt)
    # bf16 mask with one zero pad column (chain ops run even widths for 2x mode)
    m_bf = singles.tile([BPC, S + 1], bf16)
    nc.vector.memset(m_bf[:, S : S + 1], 0.0)
    nc.scalar.activation(out=m_bf[:, 0:S], in_=m_sb[:, :], func=Act.Copy)
    # pre-touch idx on GPSIMD so no gather waits on its load DMA
    idx_scr = singles.tile([16, 1], mybir.dt.uint16)
    nc.gpsimd.tensor_copy(out=idx_scr[:, :], in_=idx_sb[0:16, 0:1])

    # Z accumulator: col = ch*(NGRP*TL) + k*TL + tl; value = sum_c y+EPS over
    # the [128, C] row-block (partition p=(c,tau) holds seq 8k+c, t strip tau).
    zb = singles.tile([128, NCHUNK * NGRP * TL], f32)
    # 0/1 selector for the partition-axis reduction via PE: csel[p, m] = [p//16 == m]
    csel = singles.tile([128, 8], f32)
    nc.sync.dma_start(out=csel[:, :], in_=cselt)

    # --- producers: y strips -> Z sums + gathers -> b-major transpose ----
    # All bulk DMAs go through SWDGE (nc.gpsimd.dma_start): the HW DGE ring
    # executes dynamic DMAs nearly serially with one SDMA engine per DMA
    # (~14 GB/s), while SWDGE spreads each DMA's descriptors across all 16
    # engines.  The transpose also casts f32 -> bf16 in flight (SWDGE only).
    gtiles = []
    gdtiles = []
    ytiles = {}

    def ygen(ch, k):
        t0 = ch * TC
        ytile = ypool.tile([128, TL * C], f32, tag="ychunk")
        # src [8 seqs, 16 tau, 4KB] -> [128, 1024]: flat iteration orders
        # match (partition p = 16*c + tau), so the rank mismatch is fine.
        nc.gpsimd.dma_start(
            out=ytile[:, :],
            in_=y[8 * k : 8 * k + 8, t0 : t0 + TC, :].rearrange(
                "b (tau tl) c -> b tau (tl c)", tau=NTAU
            ),
        )
        ytiles[(ch, k)] = ytile

    YAHEAD = 2
    for j in range(YAHEAD):
        ygen(0, j)
    for ch in range(NCHUNK):
        gtile = gpool.tile([BPC, TC * SROW], bf16, tag="gchunk")
        for k in range(NGRP):
            ytile = ytiles.pop((ch, k))
            for tl in range(TL):
                scr = zscr.tile([128, C], f32, tag="zscratch")
                nc.scalar.activation(
                    out=scr[:, :],
                    in_=ytile[:, tl * C : (tl + 1) * C],
                    func=Act.Copy,
                    bias=EPS,
                    accum_out=zb[
                        :,
                        ch * NGRP * TL + k * TL + tl : ch * NGRP * TL + k * TL + tl + 1,
                    ],
                )
            gout = gopool.tile([128, NIDX], f32, tag="gout")
            for h in range(NGH):
                # Absorb Tile's sync waits (DMA RAW on ytile, WAR on gout) into
                # a cheap same-engine op: the IndirectCopy ISA struct has too
                # few sync-wait slots for the generated waits.
                nc.gpsimd.tensor_copy(out=gout[0:16, 0:1], in_=ytile[0:16, 0:1])
                nc.gpsimd.indirect_copy(
                    gout[:, h * NIDXH : (h + 1) * NIDXH],
                    ytile[:, :],
                    idx_sb[:, (NGH * k + h) * IDXC : (NGH * k + h + 1) * IDXC],
                    True,
                )
            # strip transpose + downcast: src partition 16c+tau row -> dest
            # partition 8k+c at col tau*NIDX; dest partitions sweep all 128
            # across k so every SDMA engine gets a share.
            nc.gpsimd.dma_start(out=gtile[8 * k : 8 * k + 8, :], in_=gout[:, :])
            nxt_j = ch * NGRP + k + YAHEAD
            if nxt_j < NCHUNK * NGRP:
                ygen(nxt_j // NGRP, nxt_j % NGRP)
        gtiles.append(gtile)

    # --- the DP chain -----------------------------------------------------
    SP = S + 3  # 164: cols 0,1 zero-pad, states at 2..162, col 163 junk pad
    qa = singles.tile([BPC, SP], bf16)
    qb = singles.tile([BPC, SP], bf16)
    SE = S + 1  # 162: ops run even widths so bf16 2x mode engages
    xt = singles.tile([BPC, SE], bf16)
    yt = singles.tile([BPC, SE], bf16)
    ut = singles.tile([BPC, SE], bf16)
    zst = singles.tile([BPC, NREN], f32)

    nc.vector.memset(qa[:, :], 0.0)
    nc.vector.memset(qb[:, :], 0.0)
    # q0 = g_0 at s in {0, 1}
    nc.vector.tensor_copy(out=qa[:, 2:4], in_=gtiles[0][:, 0:2])

    cur, nxt = qa, qb
    rz = None
    for t in range(1, THALF):
        ch, toff = divmod(t, TC)
        FD = min(SE, 2 * t + 2)  # band (even): q_t[s] == 0 for s > 2t+1
        g_slice = gtiles[ch][:, toff * SROW : toff * SROW + FD]
        nc.vector.tensor_tensor(
            out=yt[:, 0:FD], in0=m_bf[:, 0:FD], in1=cur[:, 0:FD], op=Alu.mult
        )
        nc.vector.tensor_tensor(
            out=xt[:, 0:FD], in0=cur[:, 2 : 2 + FD], in1=cur[:, 1 : 1 + FD], op=Alu.add
        )
        nc.vector.tensor_tensor(
            out=ut[:, 0:FD], in0=xt[:, 0:FD], in1=yt[:, 0:FD], op=Alu.add
        )
        if rz is not None:
            nc.vector.scalar_tensor_tensor(
                out=nxt[:, 2 : 2 + FD],
                in0=ut[:, 0:FD],
                scalar=rz[:, :],
                in1=g_slice,
                op0=Alu.mult,
                op1=Alu.mult,
            )
            rz = None
        else:
            nc.vector.tensor_tensor(
                out=nxt[:, 2 : 2 + FD], in0=ut[:, 0:FD], in1=g_slice, op=Alu.mult
            )
        r = t // RENORM
        if t % RENORM == RENORM - 1 and r < NREN:
            nc.vector.reduce_max(out=zst[:, r : r + 1], in_=nxt[:, 2 : 2 + S], axis=X)
            rzt = small.tile([BPC, 1], f32, tag="rz")
            nc.vector.reciprocal(out=rzt[:, :], in_=zst[:, r : r + 1])
            rz = rzt
        cur, nxt = nxt, cur

    # u-extension: virtual step THALF without the g multiply
    nc.vector.tensor_tensor(
        out=yt[:, 0:S], in0=m_bf[:, 0:S], in1=cur[:, 0:S], op=Alu.mult
    )
    nc.vector.tensor_tensor(
        out=xt[:, 0:S], in0=cur[:, 2 : 2 + S], in1=cur[:, 1 : 1 + S], op=Alu.add
    )
    nc.vector.tensor_tensor(
        out=ut[:, 0:S], in0=xt[:, 0:S], in1=yt[:, 0:S], op=Alu.add
    )
    ufs = finp.tile([BPC, S], f32, tag="fout")
    nc.scalar.activation(out=ufs[:, :], in_=ut[:, 0:S], func=Act.Copy)
    qfs = finp.tile([BPC, S], f32, tag="fout")
    nc.scalar.activation(out=qfs[:, :], in_=cur[:, 2 : 2 + S], func=Act.Copy)
    nc.sync.dma_start(out=uft, in_=ufs[:, :])
    nc.sync.dma_start(out=qft, in_=qfs[:, :])

    # --- W = sum_t ln Z, reduced to [8 c, 16 k] via PE --------------------
    lnzb = singles.tile([128, NCHUNK * NGRP * TL], f32)
    nc.scalar.activation(out=lnzb[:, :], in_=zb[:, :], func=Act.Ln)
    r1 = singles.tile([128, NGRP * NCHUNK], f32)  # [k, ch]
    lnv = lnzb[:, :].rearrange("p (ch k tl) -> p k ch tl", ch=NCHUNK, k=NGRP)
    r1v = r1[:, :].rearrange("p (k ch) -> p k ch", k=NGRP)
    nc.vector.reduce_sum(out=r1v, in_=lnv, axis=X)
    vsum = singles.tile([128, NGRP], f32)
    nc.vector.reduce_sum(
        out=vsum[:, :], in_=r1[:, :].rearrange("p (k ch) -> p k ch", k=NGRP), axis=X
    )
    psw = psump.tile([8, NGRP], f32)
    nc.tensor.matmul(psw[:, :], lhsT=csel[:, :], rhs=vsum[:, :], start=True, stop=True)
    wqs = finp.tile([8, NGRP], f32, tag="fin")
    nc.vector.tensor_copy(out=wqs[:, :], in_=psw[:, :])
    nc.sync.dma_start(out=wqt, in_=wqs[:, :])

    # --- R = sum ln z over renorms ---------------------------------------
    lnz2 = finp.tile([BPC, NREN], f32, tag="lnz")
    nc.scalar.activation(out=lnz2[:, :], in_=zst[:, :], func=Act.Ln)
    rs = finp.tile([BPC, 1], f32, tag="fin")
    nc.vector.reduce_sum(out=rs[:, :], in_=lnz2[:, :], axis=X)
    nc.sync.dma_start(out=rst, in_=rs[:, :])


@functools.lru_cache(maxsize=2)
def _build():
    from contextlib import ExitStack

    import concourse.bacc as bacc
    import concourse.mybir as mybir
    import concourse.tile as tile

    nc = bacc.Bacc(trn_type="TRN2", target_bir_lowering=False)
    y = nc.dram_tensor("y", [BPC, THALF, C], mybir.dt.float32, kind="ExternalInput")
    idxt = nc.dram_tensor(
        "idx", [128, NGRP * NGH * IDXC], mybir.dt.uint16, kind="ExternalInput"
    )
    maskt = nc.dram_tensor("mask", [BPC, S], mybir.dt.float32, kind="ExternalInput")
    cselt = nc.dram_tensor("csel", [128, 8], mybir.dt.float32, kind="ExternalInput")
    qft = nc.dram_tensor("qf", [BPC, S], mybir.dt.float32, kind="ExternalOutput")
    uft = nc.dram_tensor("uf", [BPC, S], mybir.dt.float32, kind="ExternalOutput")
    wqt = nc.dram_tensor("wq", [8, NGRP], mybir.dt.float32, kind="ExternalOutput")
    rst = nc.dram_tensor("rsum", [BPC, 1], mybir.dt.float32, kind="ExternalOutput")
    with tile.TileContext(nc) as tc:
        with ExitStack() as ctx:
            _emit_kernel(
                ctx, tc, y[:, :, :], idxt[:, :], maskt[:, :], cselt[:, :],
                qft[:, :], uft[:, :], wqt[:, :], rst[:, :],
            )
    nc.compile()
    return nc


def _wrap_idx(ext_core):
    """ext_core: [BPC, S] gather classes for one core. Returns the wrapped
    uint16 index table [128, NGRP*NGH*IDXC] for the strip gathers."""
    idx = np.zeros((128, NGRP * NGH * IDXC), dtype=np.uint16)
    js = np.arange(NIDXH)
    rows = js % 16
    cols = js // 16
    TLH = TL // NGH
    lst = np.zeros(NIDXH, dtype=np.uint16)
    for k in range(NGRP):
        for c in range(8):
            b = 8 * k + c
            for h in range(NGH):
                for tlh in range(TLH):
                    tl = h * TLH + tlh
                    lst[tlh * SROW : tlh * SROW + S] = tl * C + ext_core[b]
                    lst[tlh * SROW + S] = 0
                idx[16 * c + rows, (NGH * k + h) * IDXC + cols] = lst
    return idx


def _host_prep(y_true):
    """Per-core gather index tables and skip-transition masks (fwd + bwd)."""
    y_true = np.asarray(y_true).astype(np.int64)
    ext = np.full((B, S), BLANK, dtype=np.int64)
    ext[:, 1::2] = y_true
    mask = np.zeros((B, S), dtype=np.float32)
    mask[:, 1] = 1.0
    mask[:, 3::2] = (y_true[:, 1:] != y_true[:, :-1]).astype(np.float32)
    # backward (time+state reversed) tables
    ext_b = ext[:, ::-1]
    mask_b = np.ones((B, S), dtype=np.float32)
    sig = np.arange(2, S)
    mask_b[:, 2:] = mask[:, S + 1 - sig]
    idxs, masks = [], []
    for i in range(NPAIRS):
        b0 = i * BPC
        idxs.append(_wrap_idx(ext[b0 : b0 + BPC]))
        masks.append(np.ascontiguousarray(mask[b0 : b0 + BPC]))
    for i in range(NPAIRS):
        b0 = i * BPC
        idxs.append(_wrap_idx(ext_b[b0 : b0 + BPC]))
        masks.append(np.ascontiguousarray(mask_b[b0 : b0 + BPC]))
    return idxs, masks


def kernel(y_true, y_pred):
    from concourse.bass_utils import run_bass_kernel_spmd

    y_pred = np.asarray(y_pred, dtype=np.float32)
    idxs, masks = _host_prep(y_true)
    csel = np.zeros((128, 8), dtype=np.float32)
    csel[np.arange(128), np.arange(128) // 16] = 1.0

    nc = _build()
    in_maps = []
    for i in range(NPAIRS):  # forward cores: t in [0, 256)
        b0 = i * BPC
        in_maps.append(
            {
                "y": np.ascontiguousarray(y_pred[b0 : b0 + BPC, :THALF]),
                "idx": idxs[i],
                "mask": masks[i],
                "csel": csel,
            }
        )
    for i in range(NPAIRS):  # backward cores: t flipped from [256, 512)
        b0 = i * BPC
        in_maps.append(
            {
                "y": np.ascontiguousarray(y_pred[b0 : b0 + BPC, THALF:][:, ::-1]),
                "idx": idxs[NPAIRS + i],
                "mask": masks[NPAIRS + i],
                "csel": csel,
            }
        )
    res = run_bass_kernel_spmd(
        nc,
        in_maps,
        core_ids=list(range(NCORES)),
        trace=bool(int(os.environ.get("CTC_TRACE", "0"))),
    )
    bb = np.arange(BPC)
    out = np.empty((B, 1), dtype=np.float32)
    for i in range(NPAIRS):
        rf = res.results[i]
        rb = res.results[NPAIRS + i]
        w_f = rf["wq"][bb % 8, bb // 8].astype(np.float64)
        w_b = rb["wq"][bb % 8, bb // 8].astype(np.float64)
        dot = np.sum(
            rf["uf"].astype(np.float64) * rb["qf"][:, ::-1].astype(np.float64), axis=1
        )
        loss = (
            (w_f + w_b)
            - (rf["rsum"][:, 0].astype(np.float64) + rb["rsum"][:, 0].astype(np.float64))
            - np.log(dot)
        )
        out[i * BPC : (i + 1) * BPC, 0] = loss.astype(np.float32)
    if res.exec_time_ns is not None:
        print(f"HW exec time: {res.exec_time_ns} ns", file=sys.stderr)
    return out
